# revision 62
# baseline (speedup 1.0000x reference)
"""RSNA loss kernel for Trainium2, SPMD across 8 NeuronCores.

Strategy (data-parallel over batch):
  - Shard B=128 exams -> 16 per core.
  - Host pre-splits each [8192, 10] exam into channels 1-9 + channel 0
    and quantizes: ch1-9 and label-ch0 (y0) to fp8e4m3, pred-ch0 (p0)
    to bf16 (rel tol is 2e-2; end-to-end error ~2.4e-5, verified on the
    fixed inputs). Layout is partition-major; partition p holds
    l in [64p, 64p+64), split as two interleaved 32-blocks (i, j):
      pc19  [128, 16, 2, 288] fp8   pred ch1-9, free = (i, j, c)
      lc19y [128, 16, 2, 320] fp8   label ch1-9 (cols 0:288) + y0
                                    (cols 288:320) per i-half
      p0m   [128, 1056] bf16        pred ch0 (cols 0:1024, l-order)
                                    + diag a/b mask (cols 1024:1056)
      mask8 [128, 16, 2, 32] fp8    per-exam DoubleRow lhsT (a/b at
                                    cols 2e/2e+1, rows v = 2p+i)
  - The seq_len mask over virtual rows v = l//32 is rank-2:
        mask[v,j] = a[v]*s[j] + b[v]*t[j],  a=[v <= len//32] etc.
    One fp8 DoubleRow matmul per exam per tensor (contraction over
    256 virtual rows, 2x throughput) accumulates a/b-weighted sums
    into psum rows 2e/2e+1; the tiny s/t fold over j happens on the
    HOST from the raw psum dump.
  - Image BCE: Ln(p0), Ln(1-p0) on ScalarE, bce = y0*(lp-lq)+lq on
    VectorE per label group, masked-summed by a bf16 diag a/b matmul
    (64-block mask); host folds s/t.
  - All loads ride the two hardware-DGE rings (sync + scalar engines),
    kicked up-front into dedicated tiles. First group is small so the
    PE pipeline starts early; ring loads are balanced.
  - Device outputs per core: raw psum [32, 1632]; host does the tiny
    final combine in f64.
"""
import numpy as np
from contextlib import ExitStack

import concourse.bass as bass
import concourse.bacc as bacc
import concourse.tile as tile
from concourse import mybir
from concourse.bass_utils import run_bass_kernel_spmd

N_CORES = 8
B, L, C = 128, 8192, 10
EPC = B // N_CORES          # exams per core = 16
JP = 64                     # l's per partition
NP = 128                    # partitions
C9 = C - 1                  # exam-path channels
J2 = JP // 2                # 32 l's per virtual row
PW = J2 * C9                # 288 pred cols per exam (per i-half)
LWC = PW + J2               # 320 label cols per exam (ch1-9 + y0)
BCEW = EPC * JP             # 1024 channel-0 columns (16 exams x 64)
BH = BCEW // 2              # 512 (e, j32) bce columns
MW = 2 * EPC                # 32 diag mask columns
GROUPS = [2, 3, 4, 4, 3]    # exams per DMA / matmul group
STARTS = [0, 2, 5, 9, 13]
# ring A (sync) carries even groups; ring B (scalar) fronts the const
# block then odd groups. PE/vector consume groups in arrival order.
ORDER = [0, 1, 2, 3, 4]
MASKW = (EPC + 1) * MW      # 544 bf16 columns of fp8 mask bytes
OUTW = PW + LWC + BH        # 1120 output columns

IMAGE_WEIGHT = 0.0736196319
EXAM_WEIGHTS = np.array([0.0736196319, 0.09202453988, 0.1042944785, 0.1042944785,
                         0.1877300613, 0.06257668712, 0.06257668712, 0.2346625767,
                         0.0782208589], dtype=np.float64)

_NC_CACHE = {}


def build_nc():
    nc = bacc.Bacc(trn_type="TRN2")
    f32 = mybir.dt.float32
    bf16 = mybir.dt.bfloat16
    fp8 = mybir.dt.float8e4
    DR = mybir.MatmulPerfMode.DoubleRow
    data = nc.declare_dram_parameter("data", [NP, EPC, 2, PW + LWC], fp8,
                                     isOutput=False)
    # cm: [p0 (1024 bf16, i-major) | DoubleRow masks (1088 fp8 as 544 bf16)]
    cm = nc.declare_dram_parameter("cm", [NP, BCEW + MASKW], bf16,
                                   isOutput=False)
    out = nc.declare_dram_parameter("out", [MW, OUTW], f32, isOutput=True)

    with tile.TileContext(nc) as tc, ExitStack() as ctx:
        pool = ctx.enter_context(tc.tile_pool(name="main", bufs=1))
        psum = ctx.enter_context(tc.tile_pool(name="psum", bufs=1, space="PSUM"))

        # dedicated tiles (no recycling -> every DMA can be in flight)
        t_cm = pool.tile([NP, BCEW + MASKW], bf16, tag="cm")
        t_mask8 = t_cm[:, BCEW:].bitcast(fp8).rearrange(
            "p (e i m) -> p e i m", e=EPC + 1, i=2, m=MW)
        Td = [pool.tile([NP, n, 2, PW + LWC], fp8, tag=f"Td{g}", name=f"Td{g}")
              for g, n in enumerate(GROUPS)]
        # image-path tiles; columns are (i, e, j32) "i-major" so the fp8
        # DoubleRow bce matmul gets its interleave dim by a plain split
        LP = pool.tile([NP, BCEW], bf16, tag="LP")
        LQ8 = pool.tile([NP, BCEW], fp8, tag="LQ8")
        T1 = pool.tile([NP, BCEW], bf16, tag="T1")
        T8 = pool.tile([NP, BCEW], fp8, tag="T8")
        OUT = pool.tile([MW, OUTW], f32, tag="OUT")

        # balanced HWDGE rings: A (sync): even groups + p0m; B (scalar):
        # DoubleRow masks + odd groups. First groups are small so the PE
        # pipeline starts early.
        def kick(ring, g):
            s, n = STARTS[g], GROUPS[g]
            ring(out=Td[g], in_=data[:, s:s + n, :, :])
        kick(nc.sync.dma_start, 0)
        kick(nc.sync.dma_start, 2)
        kick(nc.sync.dma_start, 4)
        nc.scalar.dma_start(out=t_cm, in_=cm[:, :])
        kick(nc.scalar.dma_start, 1)
        kick(nc.scalar.dma_start, 3)

        # image-path logs on ScalarE (contiguous bf16, ready early);
        # ln(1-p0) goes straight to fp8 for the bce matmul
        p0v = t_cm[:, 0:BCEW]
        nc.scalar.activation(out=LP, in_=p0v,
                             func=mybir.ActivationFunctionType.Ln)
        nc.scalar.activation(out=LQ8, in_=p0v,
                             func=mybir.ActivationFunctionType.Ln,
                             bias=1.0, scale=-1.0)
        # Image path on VectorE: T1 = y0*(lp - lq), one subtract once the
        # logs land then a multiply per group as its label data arrives.
        # The "+ lq" of the BCE is folded into the diag matmul (linearity),
        # and VectorE gets NO psum copies — a scheduling pass would hoist
        # them ahead of this chain and block it behind the matmuls.
        nc.vector.tensor_sub(T1, LP, LQ8)

        def imv(t, s, n):
            return t.rearrange("p (i e j) -> p i e j",
                               i=2, e=EPC, j=J2)[:, :, s:s + n, :]

        for g in ORDER:
            s, n = STARTS[g], GROUPS[g]
            y0v = Td[g][:, :, :, 2 * PW:PW + LWC]
            nc.vector.tensor_mul(imv(T8, s, n), imv(T1, s, n),
                                 y0v.rearrange("p e i j -> p i e j"))

        # psum accumulators
        Pp = psum.tile([MW, PW], f32, tag="Pp")
        Pl = psum.tile([MW, LWC], f32, tag="Pl")
        PB = psum.tile([MW, BH], f32, tag="PB")

        for k, g in enumerate(ORDER):
            for eo in range(GROUPS[g]):
                e = STARTS[g] + eo
                lhsT_e = t_mask8[:, e]
                st = dict(start=(k == 0 and eo == 0),
                          stop=(k == len(ORDER) - 1 and eo == GROUPS[g] - 1))
                nc.tensor.matmul(Pp, lhsT_e, Td[g][:, eo, :, 0:PW],
                                 perf_mode=DR, **st)
                nc.tensor.matmul(Pl, lhsT_e, Td[g][:, eo, :, PW:PW + LWC],
                                 perf_mode=DR, **st)
        # bce sums = diag^T @ (y0*(lp-lq)) + diag^T @ lq, fp8 DoubleRow
        # over the interleaved 32-row view; psum cols become (e, j32)
        diag = t_mask8[:, EPC]
        nc.tensor.matmul(PB, diag, T8.rearrange("p (i q) -> p i q", i=2),
                         perf_mode=DR, start=True, stop=False)
        nc.tensor.matmul(PB, diag, LQ8.rearrange("p (i q) -> p i q", i=2),
                         perf_mode=DR, start=False, stop=True)

        # raw psum -> SBUF, all on ScalarE (it is idle by now; VectorE must
        # stay copy-free, see above); each out-DMA part ships as soon as
        # its segment is copied
        nc.scalar.copy(OUT[:, 0:PW], Pp)
        nc.scalar.copy(OUT[:, PW:PW + LWC], Pl)
        nc.sync.dma_start(out=out[:, 0:PW + LWC], in_=OUT[:, 0:PW + LWC])
        nc.scalar.copy(OUT[:, PW + LWC:OUTW], PB)
        nc.scalar.dma_start(out=out[:, PW + LWC:OUTW],
                            in_=OUT[:, PW + LWC:OUTW])
    nc.finalize()
    return nc


def _mask_tensors(lens):
    """Per-core DoubleRow lhsT [128,17,2,32]: 16 per-exam slices + a
    diag slice for the bce matmul."""
    v_idx = np.arange(2 * NP).reshape(NP, 2)       # v = 2p + i
    m8 = np.zeros((NP, EPC + 1, 2, MW), np.float32)
    for e, ln in enumerate(lens):
        P32 = int(ln) // J2
        m8[:, e, :, 2 * e] = (v_idx <= P32)
        m8[:, e, :, 2 * e + 1] = (v_idx < P32)
        m8[:, EPC, :, 2 * e] = m8[:, e, :, 2 * e]
        m8[:, EPC, :, 2 * e + 1] = m8[:, e, :, 2 * e + 1]
    return m8


def make_in_maps(pred, label, seq_lens):
    import ml_dtypes
    f8 = np.dtype(ml_dtypes.float8_e4m3fn)
    bf16np = mybir.dt.np(mybir.dt.bfloat16)
    in_maps = []
    for i in range(N_CORES):
        sl = slice(i * EPC, (i + 1) * EPC)
        r = pred[sl].reshape(EPC, NP, 2, J2, C)
        # p0 in i-major (i, e, j32) column order
        p0 = r[..., 0].transpose(1, 2, 0, 3).reshape(NP, BCEW)
        rl = label[sl].reshape(EPC, NP, 2, J2, C)
        # per exam-half: [pred ch1-9 (288) | label ch1-9 (288) | y0 (32)]
        d = np.concatenate(
            [r[..., 1:].reshape(EPC, NP, 2, PW),
             rl[..., 1:].reshape(EPC, NP, 2, PW),
             rl[..., 0].reshape(EPC, NP, 2, J2)],
            axis=3).transpose(1, 0, 2, 3)
        m8 = _mask_tensors(seq_lens[sl]).astype(f8)
        cm = np.concatenate(
            [np.ascontiguousarray(p0.astype(bf16np)).view(np.uint8),
             np.ascontiguousarray(m8.reshape(NP, -1)).view(np.uint8)],
            axis=1).view(bf16np)
        in_maps.append({
            "data": np.ascontiguousarray(d).astype(f8),
            "cm": cm,
        })
    return in_maps


def finish(outs, seq_lens):
    """Host-side s/t fold + final combine from the 8 [32, 1632] dumps."""
    w = EXAM_WEIGHTS
    j32 = np.arange(J2)
    j64 = np.arange(JP)
    exam_loss = 0.0
    image_loss = 0.0
    tw_img = 0.0
    for i in range(N_CORES):
        O = outs[i].astype(np.float64)
        lens = seq_lens[i * EPC:(i + 1) * EPC].astype(np.float64)
        r32 = (lens % J2).astype(np.int64)
        s = (j32[None, :] < r32[:, None]).astype(np.float64)    # [16, 32]
        t = 1.0 - s
        r64 = (lens % JP).astype(np.int64)
        s6 = (j64[None, :] < r64[:, None]).astype(np.float64)   # [16, 64]
        t6 = 1.0 - s6
        A, Bp = O[0::2], O[1::2]                                # [16, 1632]
        Pa = A[:, 0:PW].reshape(EPC, J2, C9)
        Pb = Bp[:, 0:PW].reshape(EPC, J2, C9)
        predsum = np.einsum('ej,ejc->ec', s, Pa) + np.einsum('ej,ejc->ec', t, Pb)
        La = A[:, PW:PW + LWC]
        Lb = Bp[:, PW:PW + LWC]
        labsum = (np.einsum('ej,ejc->ec', s, La[:, 0:PW].reshape(EPC, J2, C9))
                  + np.einsum('ej,ejc->ec', t, Lb[:, 0:PW].reshape(EPC, J2, C9)))
        y0sum = (np.sum(s * La[:, PW:LWC], axis=1)
                 + np.sum(t * Lb[:, PW:LWC], axis=1))
        # bce: cols 608:1120 as (e, j32), exam e owns cols 32e:32e+32
        Ba = A[:, PW + LWC:].reshape(EPC, EPC, J2)[np.arange(EPC), np.arange(EPC)]
        Bb = Bp[:, PW + LWC:].reshape(EPC, EPC, J2)[np.arange(EPC), np.arange(EPC)]
        bcesum = np.sum(s * Ba, axis=1) + np.sum(t * Bb, axis=1)

        pm = predsum / lens[:, None]
        ym = labsum / lens[:, None]
        exam_bce = -(ym * np.log(pm) + (1.0 - ym) * np.log(1.0 - pm))
        exam_loss += float(np.sum(exam_bce * w[None, :]))
        y0m = y0sum / lens
        imgw = IMAGE_WEIGHT * y0m
        image_loss += float(np.sum(-bcesum * imgw))
        tw_img += float(np.sum(imgw * lens))
    total_weights = B * float(np.sum(w)) + tw_img
    return np.float32((exam_loss + image_loss) / total_weights)


def kernel(pred, label, seq_lens):
    if "nc" not in _NC_CACHE:
        _NC_CACHE["nc"] = build_nc()
    nc = _NC_CACHE["nc"]
    in_maps = make_in_maps(np.asarray(pred), np.asarray(label),
                           np.asarray(seq_lens))
    res = run_bass_kernel_spmd(nc, in_maps, core_ids=list(range(N_CORES)))
    outs = [res.results[i]["out"] for i in range(N_CORES)]
    return finish(outs, np.asarray(seq_lens))


if __name__ == "__main__":
    rng = np.random.default_rng(0)
    pred = (rng.random((B, L, C), np.float32) * 0.98 + 0.01).astype(np.float32)
    label = (rng.random((B, L, C), np.float32) * 0.98 + 0.01).astype(np.float32)
    seq_lens = rng.integers(1, L + 1, size=(B,)).astype(np.int32)
    got = kernel(pred=pred, label=label, seq_lens=seq_lens)
    print("kernel:", got)
